# revision 1
# baseline (speedup 1.0000x reference)
"""Trainium2 Bass kernel for nn_MultiHeadAttention (B=2, S=2048, D=1024, H=16).

Sharding: 8 cores = 2 batches x 4 head-groups (4 heads / 256 dims each).
Each core computes its head-group's QKV projections, attention, and a
partial output projection (Megatron row-parallel); host sums the 4
partials per batch and adds the bias terms.

Layouts: host pre-transposes activations/weights so every matmul operand
is DMA-able with the contraction dim on SBUF partitions. All matmuls run
as float32r (TF32-like, 11-bit mantissa, full PE speed); inputs are
pre-rounded on host so device results are deterministic.
"""
import sys
sys.path.insert(0, '/opt/trn_rl_repo')

from contextlib import ExitStack

import numpy as np

import concourse.bass as bass
import concourse.mybir as mybir
import concourse.tile as tile
from concourse import bacc
from concourse.bass_utils import run_bass_kernel_spmd

B, S, D, H = 2, 2048, 1024, 16
HD = D // H            # 64
NCORES = 8
GROUPS = 4             # head groups (tensor parallel)
DL = D // GROUPS       # 256 local d_out per core
HL = H // GROUPS       # 4 local heads
P = 128
KC = S // P            # 16 k-chunks
SC = D // P            # 8 d_in chunks
NSLAB = 4              # s-slabs of 512 for projections
F32R = mybir.dt.float32r
F32 = mybir.dt.float32


def _round_f32r(x):
    """Round fp32 to float32r (11 explicit mantissa bits, round-to-nearest)."""
    xi = np.ascontiguousarray(x, np.float32).view(np.uint32).astype(np.uint64)
    add = np.uint64(1 << 11)
    mask = np.uint64(0xFFFFFFFFFFFFF000)
    return ((xi + add) & mask).astype(np.uint32).view(np.float32)


def _build_module():
    nc = bacc.Bacc(None, target_bir_lowering=False, debug=False)

    qT = nc.dram_tensor("qT", [D, S], F32R, kind="ExternalInput").ap()
    kT = nc.dram_tensor("kT", [D, S], F32R, kind="ExternalInput").ap()
    vT = nc.dram_tensor("vT", [D, S], F32R, kind="ExternalInput").ap()
    wqT = nc.dram_tensor("wqT", [D, DL], F32R, kind="ExternalInput").ap()
    wkT = nc.dram_tensor("wkT", [D, DL], F32R, kind="ExternalInput").ap()
    wvT = nc.dram_tensor("wvT", [D, DL], F32R, kind="ExternalInput").ap()
    woT = nc.dram_tensor("woT", [DL, D], F32R, kind="ExternalInput").ap()
    bq2 = nc.dram_tensor("bq2", [2, P], F32, kind="ExternalInput").ap()
    bk2 = nc.dram_tensor("bk2", [2, P], F32, kind="ExternalInput").ap()
    out = nc.dram_tensor("out", [S, D], F32, kind="ExternalOutput").ap()

    qTv = qT.rearrange("(kc p) s -> p kc s", p=P)
    kTv = kT.rearrange("(kc p) s -> p kc s", p=P)
    vTv = vT.rearrange("(kc p) s -> p kc s", p=P)

    with tile.TileContext(nc) as tc:
        with ExitStack() as ctx:
            wpool = ctx.enter_context(tc.tile_pool(name="weights", bufs=1))
            big = ctx.enter_context(tc.tile_pool(name="big", bufs=1))
            slab = ctx.enter_context(tc.tile_pool(name="slab", bufs=2))
            ptp = ctx.enter_context(tc.tile_pool(name="pt", bufs=3))
            recp = ctx.enter_context(tc.tile_pool(name="rec", bufs=2))
            outp = ctx.enter_context(tc.tile_pool(name="outsb", bufs=2))

            # ---- weights / constants ----
            wq_sb = wpool.tile([P, SC, DL], F32R)
            wk_sb = wpool.tile([P, SC, DL], F32R)
            wv_sb = wpool.tile([P, SC, DL], F32R)
            wo_sb = wpool.tile([P, DL // P, D], F32R)
            bq_sb = wpool.tile([P, 2], F32)
            bk_sb = wpool.tile([P, 2], F32)
            ones_sb = wpool.tile([P, HD], F32)
            nc.sync.dma_start(wq_sb[:], wqT.rearrange("(kc p) m -> p kc m", p=P))
            nc.sync.dma_start(wk_sb[:], wkT.rearrange("(kc p) m -> p kc m", p=P))
            nc.sync.dma_start(wv_sb[:], wvT.rearrange("(kc p) m -> p kc m", p=P))
            nc.sync.dma_start(wo_sb[:], woT.rearrange("(c p) n -> p c n", p=P))
            nc.sync.dma_start(bq_sb[:], bq2.rearrange("m p -> p m"))
            nc.sync.dma_start(bk_sb[:], bk2.rearrange("m p -> p m"))
            nc.gpsimd.memset(ones_sb[:], 1.0)

            # ---- persistent activations ----
            QT = big.tile([P, 2, S], F32R)          # [d_out within pair, m-chunk, q]
            KT = big.tile([P, 2, S], F32R)
            V2 = big.tile([P, KC, HL, 2 * HD], F32R)  # [k, chunk, head, V|ones]
            xT = big.tile([P, 2, S], F32R)          # attention out, transposed

            nc.vector.tensor_copy(
                V2[:, :, :, HD:2 * HD],
                ones_sb[:, None, None, :].to_broadcast([P, KC, HL, HD]),
            )

            # ---- phase 2: projections ----
            with tc.tile_pool(name="proj_ps", bufs=2, space="PSUM") as proj_ps, \
                 tc.tile_pool(name="projv_ps", bufs=2, space="PSUM") as projv_ps:
                for j in range(NSLAB):
                    js = slice(j * 512, (j + 1) * 512)
                    qslab = slab.tile([P, SC, 512], F32R, tag="slab")
                    nc.sync.dma_start(qslab[:], qTv[:, :, js])
                    for m in range(2):
                        ps = proj_ps.tile([P, 512], F32)
                        for kc in range(SC):
                            nc.tensor.matmul(
                                ps[:], wq_sb[:, kc, m * P:(m + 1) * P],
                                qslab[:, kc, :],
                                start=(kc == 0), stop=(kc == SC - 1))
                        nc.vector.tensor_scalar_add(
                            QT[:, m, js], ps[:], bq_sb[:, m:m + 1])

                    kslab = slab.tile([P, SC, 512], F32R, tag="slab")
                    nc.sync.dma_start(kslab[:], kTv[:, :, js])
                    for m in range(2):
                        ps = proj_ps.tile([P, 512], F32)
                        for kc in range(SC):
                            nc.tensor.matmul(
                                ps[:], wk_sb[:, kc, m * P:(m + 1) * P],
                                kslab[:, kc, :],
                                start=(kc == 0), stop=(kc == SC - 1))
                        nc.vector.tensor_scalar_add(
                            KT[:, m, js], ps[:], bk_sb[:, m:m + 1])

                    vslab = slab.tile([P, SC, 512], F32R, tag="slab")
                    nc.sync.dma_start(vslab[:], vTv[:, :, js])
                    for ss in range(4):
                        psv = projv_ps.tile([P, DL], F32)
                        for kc in range(SC):
                            nc.tensor.matmul(
                                psv[:], vslab[:, kc, ss * P:(ss + 1) * P],
                                wv_sb[:, kc, :],
                                start=(kc == 0), stop=(kc == SC - 1))
                        nc.vector.tensor_copy(
                            V2[:, j * 4 + ss, :, 0:HD],
                            psv[:].rearrange("p (h d) -> p h d", d=HD))

            # ---- phase 3: attention per head ----
            with tc.tile_pool(name="st_ps", bufs=2, space="PSUM") as st_ps, \
                 tc.tile_pool(name="av_ps", bufs=1, space="PSUM") as av_ps:
                for h in range(HL):
                    hp, hm = (h % 2) * HD, h // 2
                    av = av_ps.tile([P, S], F32, tag="av")
                    for kc in range(KC):
                        for half in range(2):
                            st = st_ps.tile([P, 1024], F32, tag="st")
                            for qq in range(2):
                                q0 = half * 1024 + qq * 512
                                nc.tensor.matmul(
                                    st[:, qq * 512:(qq + 1) * 512],
                                    KT[hp:hp + HD, hm, kc * P:(kc + 1) * P],
                                    QT[hp:hp + HD, hm, q0:q0 + 512],
                                    start=True, stop=True)
                            pt = ptp.tile([P, 1024], F32R, tag="pt")
                            nc.scalar.activation(
                                pt[:], st[:],
                                mybir.ActivationFunctionType.Exp, scale=0.125)
                            for qq in range(2):
                                q0 = half * 1024 + qq * 512
                                nc.tensor.matmul(
                                    av[:, q0:q0 + 512],
                                    V2[:, kc, h, :], pt[:, qq * 512:(qq + 1) * 512],
                                    start=(kc == 0), stop=(kc == KC - 1))
                    rec = recp.tile([HD, S], F32, tag="rec")
                    nc.vector.reciprocal(rec[:], av[HD:2 * HD, :])
                    nc.vector.tensor_tensor(
                        xT[hp:hp + HD, hm, :], av[0:HD, :], rec[:],
                        mybir.AluOpType.mult)

            # ---- phase 4: output projection (partial) ----
            with tc.tile_pool(name="op_ps", bufs=2, space="PSUM") as op_ps:
                for j in range(KC):
                    op = op_ps.tile([P, 1024], F32)
                    for n in range(2):
                        for ci in range(2):
                            nc.tensor.matmul(
                                op[:, n * 512:(n + 1) * 512],
                                xT[:, ci, j * P:(j + 1) * P],
                                wo_sb[:, ci, n * 512:(n + 1) * 512],
                                start=(ci == 0), stop=(ci == 1))
                    osb = outp.tile([P, 1024], F32, tag="osb")
                    nc.vector.tensor_copy(osb[:], op[:])
                    nc.sync.dma_start(out[j * P:(j + 1) * P, :], osb[:])

    nc.compile()
    return nc


_NC = None


def _get_nc():
    global _NC
    if _NC is None:
        _NC = _build_module()
    return _NC


def kernel(query, key, value, mask, Wq, bq, Wk, bk, Wv, bv, Wo, bo,
           _trace=False):
    query = np.asarray(query, np.float32)
    key = np.asarray(key, np.float32)
    value = np.asarray(value, np.float32)
    Wq, Wk, Wv, Wo = (np.asarray(w, np.float32) for w in (Wq, Wk, Wv, Wo))
    bq, bk, bv, bo = (np.asarray(b_, np.float32) for b_ in (bq, bk, bv, bo))
    mask = np.asarray(mask, bool)

    # host-side layout prep (shared across the 4 cores of each batch)
    qT = [_round_f32r(query[b].T) for b in range(B)]
    kTh = [_round_f32r(key[b].T) for b in range(B)]
    vTh = [_round_f32r(value[b].T) for b in range(B)]

    in_maps = []
    for c in range(NCORES):
        b, g = c // GROUPS, c % GROUPS
        gs = slice(g * DL, (g + 1) * DL)
        in_maps.append({
            "qT": qT[b], "kT": kTh[b], "vT": vTh[b],
            "wqT": _round_f32r(Wq[gs, :].T),
            "wkT": _round_f32r(Wk[gs, :].T),
            "wvT": _round_f32r(Wv[gs, :].T),
            "woT": _round_f32r(Wo[:, gs].T),
            "bq2": np.ascontiguousarray(bq[gs].reshape(2, P)),
            "bk2": np.ascontiguousarray(bk[gs].reshape(2, P)),
        })

    nc = _get_nc()
    res = run_bass_kernel_spmd(nc, in_maps, core_ids=list(range(NCORES)),
                               trace=_trace)

    extra = (bv @ Wo.T + bo).astype(np.float32)  # bv folds through out-proj
    output = np.zeros((B, S, D), np.float32)
    for c in range(NCORES):
        output[c // GROUPS] += res.results[c]["out"]
    output += extra

    # masked query rows attend uniformly (softmax of constant -1e9)
    if mask.any():
        for b in range(B):
            rows = np.nonzero(mask[b, 0])[0]
            if rows.size:
                v_full = value[b] @ Wv.T + bv
                out_row = v_full.mean(0) @ Wo.T + bo
                output[b, rows, :] = out_row

    if _trace:
        return output, res
    return output
